# revision 1
# baseline (speedup 1.0000x reference)
"""GraphSAGE-style 2-layer minibatch forward (gnn_message_passing) on 8 trn2
NeuronCores — self-contained bass/Tile kernel.

Problem (hardcoded shapes):
    features [1_000_000, 128] f32, W0/W1 [128, 256] f32, b0/b1 [128] f32,
    nodes [4096] int, nbr1 [4096, 10] int, nbr0 [4096, 11, 25] int
    out z [4096, 128] f32

Strategy: data-parallel over the 4096 target nodes (512 per core, 4 tiles of
128); the feature table and the small weights are replicated per core, so no
cross-core communication is needed.  Per (tile, slot) the kernel issues 26
indirect-DMA gathers (one table row per SBUF partition, 128 rows / 64 KB per
instruction — the only vector-indirect form the SWDGE ucode supports),
round-robined over 4 SWDGE queues.  The 25 neighbor rows are summed on DVE
(strided tensor_reduce), self/neighbor-sum tiles transposed on PE, and both
dense layers run as K-chunked PE matmuls with the bias added via a rank-1
matmul.  The mean-over-neighbors is folded into pre-scaled weight halves:
    relu(W0 @ [x_self ; mean_j x_j]) == relu(x_self @ W0s.T + (sum_j x_j) @ (W0n/25).T)
The l2-normalize uses 1/sqrt(ssq + 1e-24), which matches the reference's
h / max(||h||, 1e-12) for every reachable input.
"""
import numpy as np

N = 1_000_000
F = 128
H = 128
B = 4096
S1 = 10
S0 = 25
NC = 8
BC = B // NC          # 512 targets per core
T = 1 + S1            # 11 level-1 slots
P = 128
NTILES = BC // P      # 4

_QUEUES = ["qPoolDynamic", "qPoolDynamic1", "qPoolDynamic2", "qPoolDynamic3"]
import os as _os
_SINGLE_PACKET = _os.environ.get("K_SINGLE_PACKET", "1") == "1"
_NQUEUES = int(_os.environ.get("K_NQUEUES", "4"))
_NBUFS = int(_os.environ.get("K_NBUFS", "5"))

_CACHE = {}


# ----------------------------------------------------------------- bass path
def _build_nc():
    import concourse.bass as bass
    import concourse.tile as tile
    from contextlib import ExitStack
    from concourse import bacc, mybir
    from concourse.masks import make_identity

    nc = bacc.Bacc("TRN2", target_bir_lowering=False, debug=False,
                   num_swdge_queues=_NQUEUES)
    feat = nc.dram_tensor("features", [N, F], mybir.dt.float32, kind="ExternalInput")
    w0t = nc.dram_tensor("w0t", [2 * F, H], mybir.dt.float32, kind="ExternalInput")
    w1t = nc.dram_tensor("w1t", [2 * H, H], mybir.dt.float32, kind="ExternalInput")
    b0r = nc.dram_tensor("b0r", [1, H], mybir.dt.float32, kind="ExternalInput")
    b1r = nc.dram_tensor("b1r", [1, H], mybir.dt.float32, kind="ExternalInput")
    idx_self = nc.dram_tensor("idx_self", [BC, T], mybir.dt.int32, kind="ExternalInput")
    idx_nbr = nc.dram_tensor("idx_nbr", [BC, T * S0], mybir.dt.int32, kind="ExternalInput")
    zout = nc.dram_tensor("zout", [BC, H], mybir.dt.float32, kind="ExternalOutput")

    qi = 0

    def gather(out_ap, off_ap):
        nonlocal qi
        ins_obj = nc.gpsimd.indirect_dma_start(
            out=out_ap, out_offset=None, in_=feat[:],
            in_offset=bass.IndirectOffsetOnAxis(ap=off_ap, axis=0),
        )
        ins_obj.ins.queue = _QUEUES[qi % _NQUEUES]
        ins_obj.ins.single_packet = _SINGLE_PACKET
        qi += 1
        return ins_obj

    with tile.TileContext(nc) as tc:
        with ExitStack() as ctx:
            const = ctx.enter_context(tc.tile_pool(name="const", bufs=1))
            idxp = ctx.enter_context(tc.tile_pool(name="idxp", bufs=2))
            nbrp = ctx.enter_context(tc.tile_pool(name="nbrp", bufs=_NBUFS))
            selfp = ctx.enter_context(tc.tile_pool(name="selfp", bufs=_NBUFS))
            work = ctx.enter_context(tc.tile_pool(name="work", bufs=6))
            slab = ctx.enter_context(tc.tile_pool(name="slab", bufs=2))
            outp = ctx.enter_context(tc.tile_pool(name="outp", bufs=2))
            psum_t = ctx.enter_context(tc.tile_pool(name="psum_t", bufs=4, space="PSUM"))
            psum_a = ctx.enter_context(tc.tile_pool(name="psum_a", bufs=4, space="PSUM"))

            ident = const.tile([P, P], mybir.dt.float32)
            make_identity(nc, ident[:])
            ones = const.tile([1, P], mybir.dt.float32)
            nc.vector.memset(ones[:], 1.0)
            eps = const.tile([P, 1], mybir.dt.float32, tag="eps")
            nc.vector.memset(eps[:], 1e-24)
            w0_sb = const.tile([P, 2, H], mybir.dt.float32, tag="w0sb")
            nc.sync.dma_start(out=w0_sb[:, 0, :], in_=w0t[0:F, :])
            nc.sync.dma_start(out=w0_sb[:, 1, :], in_=w0t[F:2 * F, :])
            w1_sb = const.tile([P, 2, H], mybir.dt.float32, tag="w1sb")
            nc.sync.dma_start(out=w1_sb[:, 0, :], in_=w1t[0:H, :])
            nc.sync.dma_start(out=w1_sb[:, 1, :], in_=w1t[H:2 * H, :])
            b0_sb = const.tile([1, H], mybir.dt.float32, tag="b0sb")
            nc.sync.dma_start(out=b0_sb[:], in_=b0r.ap())
            b1_sb = const.tile([1, H], mybir.dt.float32, tag="b1sb")
            nc.sync.dma_start(out=b1_sb[:], in_=b1r.ap())

            def layer_tail(acc_psum, dest_ap):
                h = work.tile([P, H], mybir.dt.float32, tag="relu")
                nc.scalar.activation(out=h[:], in_=acc_psum,
                                     func=mybir.ActivationFunctionType.Relu)
                sq = work.tile([P, H], mybir.dt.float32, tag="sq")
                ssq = work.tile([P, 1], mybir.dt.float32, tag="ssq")
                nc.scalar.activation(out=sq[:], in_=h[:],
                                     func=mybir.ActivationFunctionType.Square,
                                     accum_out=ssq[:])
                nrm = work.tile([P, 1], mybir.dt.float32, tag="nrm")
                nc.scalar.activation(out=nrm[:], in_=ssq[:],
                                     func=mybir.ActivationFunctionType.Sqrt,
                                     bias=eps[:])
                rn = work.tile([P, 1], mybir.dt.float32, tag="rn")
                nc.vector.reciprocal(out=rn[:], in_=nrm[:])
                nc.vector.tensor_scalar_mul(dest_ap, h[:], rn[:])

            def transpose_to_sb(src_ap, tag):
                pt = psum_t.tile([P, P], mybir.dt.float32, tag="tp")
                nc.tensor.transpose(out=pt[:], in_=src_ap, identity=ident[:])
                sb = work.tile([P, P], mybir.dt.float32, tag=tag)
                nc.vector.tensor_copy(out=sb[:], in_=pt[:])
                return sb

            import concourse.mybir as mybir_  # alias for closures above
            for t in range(NTILES):
                rows = slice(t * P, (t + 1) * P)
                idxs_t = idxp.tile([P, T], mybir.dt.int32, tag="idxs")
                nc.sync.dma_start(out=idxs_t[:], in_=idx_self[rows, :])
                idxn_t = idxp.tile([P, T * S0], mybir.dt.int32, tag="idxn")
                nc.sync.dma_start(out=idxn_t[:], in_=idx_nbr[rows, :])

                h1slab = slab.tile([P, T, H], mybir.dt.float32, tag="h1")

                for s in range(T):
                    sf = selfp.tile([P, F], mybir.dt.float32, tag="sf")
                    gather(sf[:], idxs_t[:, s:s + 1])
                    g = nbrp.tile([P, S0, F], mybir.dt.float32, tag="g")
                    for j in range(S0):
                        gather(g[:, j, :], idxn_t[:, s * S0 + j:s * S0 + j + 1])

                    ns = work.tile([P, F], mybir.dt.float32, tag="ns")
                    nc.vector.tensor_reduce(
                        out=ns[:], in_=g[:].rearrange("p j f -> p f j"),
                        axis=mybir.AxisListType.X, op=mybir.AluOpType.add,
                    )
                    xa = transpose_to_sb(sf[:], "xa")
                    xb = transpose_to_sb(ns[:], "xb")
                    acc = psum_a.tile([P, H], mybir.dt.float32, tag="acc")
                    nc.tensor.matmul(acc[:], lhsT=xa[:], rhs=w0_sb[:, 0, :], start=True, stop=False)
                    nc.tensor.matmul(acc[:], lhsT=xb[:], rhs=w0_sb[:, 1, :], start=False, stop=False)
                    nc.tensor.matmul(acc[:], lhsT=ones[:1, :], rhs=b0_sb[:1, :], start=False, stop=True)
                    layer_tail(acc[:], h1slab[:, s, :])

                ns2 = work.tile([P, H], mybir.dt.float32, tag="ns2")
                nc.vector.tensor_reduce(
                    out=ns2[:], in_=h1slab[:, 1:, :].rearrange("p s f -> p f s"),
                    axis=mybir.AxisListType.X, op=mybir.AluOpType.add,
                )
                x2a = transpose_to_sb(h1slab[:, 0, :], "xa")
                x2b = transpose_to_sb(ns2[:], "xb")
                acc2 = psum_a.tile([P, H], mybir.dt.float32, tag="acc")
                nc.tensor.matmul(acc2[:], lhsT=x2a[:], rhs=w1_sb[:, 0, :], start=True, stop=False)
                nc.tensor.matmul(acc2[:], lhsT=x2b[:], rhs=w1_sb[:, 1, :], start=False, stop=False)
                nc.tensor.matmul(acc2[:], lhsT=ones[:1, :], rhs=b1_sb[:1, :], start=False, stop=True)
                zt = outp.tile([P, H], mybir.dt.float32, tag="z")
                layer_tail(acc2[:], zt[:])
                nc.sync.dma_start(out=zout[rows, :], in_=zt[:])

    nc.compile()
    return nc


def _prep_host(features, W0, b0, W1, b1, nodes, nbr1, nbr0):
    features = np.ascontiguousarray(np.asarray(features, dtype=np.float32))
    W0 = np.asarray(W0, dtype=np.float32)
    W1 = np.asarray(W1, dtype=np.float32)
    b0 = np.asarray(b0, dtype=np.float32).reshape(1, H)
    b1 = np.asarray(b1, dtype=np.float32).reshape(1, H)
    lvl1 = np.concatenate([np.asarray(nodes).reshape(B, 1),
                           np.asarray(nbr1).reshape(B, S1)], axis=1).astype(np.int32)
    nbr0f = np.asarray(nbr0).reshape(B, T * S0).astype(np.int32)
    w0t = np.ascontiguousarray(np.concatenate([W0[:, :F].T, W0[:, F:].T / S0], axis=0))
    w1t = np.ascontiguousarray(np.concatenate([W1[:, :H].T, W1[:, H:].T / S1], axis=0))
    return features, w0t, w1t, b0, b1, lvl1, nbr0f


def _kernel_bass(features, W0, b0, W1, b1, nodes, nbr1, nbr0, trace=False):
    from concourse.bass_utils import run_bass_kernel_spmd
    if "nc" not in _CACHE:
        _CACHE["nc"] = _build_nc()
    nc = _CACHE["nc"]
    features, w0t, w1t, b0, b1, lvl1, nbr0f = _prep_host(
        features, W0, b0, W1, b1, nodes, nbr1, nbr0)
    in_maps = []
    for c in range(NC):
        sl = slice(c * BC, (c + 1) * BC)
        in_maps.append(dict(
            features=features, w0t=w0t, w1t=w1t, b0r=b0, b1r=b1,
            idx_self=np.ascontiguousarray(lvl1[sl]),
            idx_nbr=np.ascontiguousarray(nbr0f[sl]),
        ))
    res = run_bass_kernel_spmd(nc, in_maps, core_ids=list(range(NC)), trace=trace)
    out = np.concatenate([res.results[c]["zout"] for c in range(NC)], axis=0)
    if trace:
        return out, res
    return out


# ----------------------------------------------------- v2: two-phase bounce
# Phase A: per (tile-half, 32K-row window) dma_gather of the half's
# row-sorted refs (padded to W_BUD) into SBUF, streamed to a compact HBM
# staging table. Phase C: dma_gather from the <=32K-row staging straight into
# the final [128, 286, F] compute layout (idx = host-computed staged
# positions). Compute identical to v1. ~390 Ant-gather instructions replace
# 1144 indirect ones on the Pool engine.
W_BUD = 768                # staged rows per (half, window) = 6 chunks
NW = 31                    # 31 windows cover rows [0, 1_015_808) >= N
HCH = T * 26 // 2          # 143 chunks per half-tile
NPOS = HCH * P             # 18304 final positions per half
STG_H = NW * W_BUD         # 23808 staged rows per half (int16-addressable)
NPADROWS = 1 << 20
AW = W_BUD // 16           # 48 idx words per window slot
CW = 64                    # idx words per C-block slot (1024/16)
NCB = 18                   # C blocks per half (17x1024 + 896)


def _wrap16(vals, width):
    a = np.zeros((16, width), np.int16)
    n = len(vals)
    a[np.arange(n) % 16, np.arange(n) // 16] = vals.astype(np.int16)
    return np.tile(a, (8, 1))


def _plan_v2(lvl1c, nbr0c3):
    """Per-core plan: (aidx [128, 8*NW*AW], cidx [128, 8*NCB*CW]) or None."""
    aidx = np.zeros((128, 8 * NW * AW), np.int16)
    cidx = np.zeros((128, 8 * NCB * CW), np.int16)
    for t in range(NTILES):
        trows = t * P + np.arange(P)
        for a in (0, 1):
            h = 2 * t + a
            ms = np.arange(a * HCH, (a + 1) * HCH)
            s_of, j_of = ms // 26, ms % 26
            r_self = lvl1c[trows[:, None], s_of[None, :]]
            r_nbr = nbr0c3[trows[:, None], s_of[None, :],
                           np.maximum(j_of - 1, 0)[None, :]]
            refs = np.where((j_of == 0)[None, :], r_self, r_nbr)   # [p, m]
            flat = refs.T.reshape(-1)                              # k = m*128+p
            order = np.argsort(flat, kind="stable")
            sref = flat[order]
            w = (sref >> 15).astype(np.int64)
            cnt = np.bincount(w, minlength=NW)
            if cnt.max() > W_BUD:
                return None, None
            starts = np.concatenate([[0], np.cumsum(cnt)])
            rank = np.arange(NPOS) - starts[w]
            # staged row within the window block follows the plain [p, c, f]
            # write order of the gather tile: token r -> row (r%128)*6 + r//128
            spos = w * W_BUD + (rank % P) * (W_BUD // P) + rank // P
            cpos = np.empty(NPOS, np.int64)
            cpos[order] = spos
            for wi in range(NW):
                loc = sref[starts[wi]:starts[wi + 1]] - (wi << 15)
                aidx[:, (h * NW + wi) * AW:(h * NW + wi + 1) * AW] = \
                    _wrap16(np.concatenate([loc, np.zeros(W_BUD - len(loc),
                                                          np.int64)]), AW)
            for b in range(NCB):
                nidx = 1024 if b < NCB - 1 else NPOS - 1024 * (NCB - 1)
                vals = cpos[b * 1024: b * 1024 + nidx]
                cidx[:, (h * NCB + b) * CW:(h * NCB + b) * CW + CW] = \
                    _wrap16(vals, CW)
    return aidx, cidx


def _build_nc_v2():
    import concourse.bass as bass
    import concourse.tile as tile
    from contextlib import ExitStack
    from concourse import bacc, mybir
    from concourse.masks import make_identity

    nc = bacc.Bacc("TRN2", target_bir_lowering=False, debug=False,
                   num_swdge_queues=4)
    feat = nc.dram_tensor("features", [NPADROWS, F], mybir.dt.float32, kind="ExternalInput")
    w0t = nc.dram_tensor("w0t", [2 * F, H], mybir.dt.float32, kind="ExternalInput")
    w1t = nc.dram_tensor("w1t", [2 * H, H], mybir.dt.float32, kind="ExternalInput")
    b0r = nc.dram_tensor("b0r", [1, H], mybir.dt.float32, kind="ExternalInput")
    b1r = nc.dram_tensor("b1r", [1, H], mybir.dt.float32, kind="ExternalInput")
    aidx_d = nc.dram_tensor("aidx", [128, 8 * NW * AW], mybir.dt.int16, kind="ExternalInput")
    cidx_d = nc.dram_tensor("cidx", [128, 8 * NCB * CW], mybir.dt.int16, kind="ExternalInput")
    stg = [nc.dram_tensor(f"stg{h}", [STG_H, F], mybir.dt.float32,
                          kind="Internal") for h in range(8)]
    zout = nc.dram_tensor("zout", [BC, H], mybir.dt.float32, kind="ExternalOutput")

    qi = 0

    with tile.TileContext(nc) as tc:
        with ExitStack() as ctx:
            const = ctx.enter_context(tc.tile_pool(name="const", bufs=1))
            aip = ctx.enter_context(tc.tile_pool(name="aip", bufs=1))
            cip = ctx.enter_context(tc.tile_pool(name="cip", bufs=2))
            stgp = ctx.enter_context(tc.tile_pool(name="stgp", bufs=4))
            finp = ctx.enter_context(tc.tile_pool(name="finp", bufs=1))
            work = ctx.enter_context(tc.tile_pool(name="work", bufs=6))
            slab = ctx.enter_context(tc.tile_pool(name="slab", bufs=2))
            outp = ctx.enter_context(tc.tile_pool(name="outp", bufs=2))
            psum_t = ctx.enter_context(tc.tile_pool(name="psum_t", bufs=4, space="PSUM"))
            psum_a = ctx.enter_context(tc.tile_pool(name="psum_a", bufs=4, space="PSUM"))

            ident = const.tile([P, P], mybir.dt.float32)
            make_identity(nc, ident[:])
            ones = const.tile([1, P], mybir.dt.float32)
            nc.vector.memset(ones[:], 1.0)
            eps = const.tile([P, 1], mybir.dt.float32, tag="eps")
            nc.vector.memset(eps[:], 1e-24)
            w0_sb = const.tile([P, 2, H], mybir.dt.float32, tag="w0sb")
            nc.sync.dma_start(out=w0_sb[:, 0, :], in_=w0t[0:F, :])
            nc.sync.dma_start(out=w0_sb[:, 1, :], in_=w0t[F:2 * F, :])
            w1_sb = const.tile([P, 2, H], mybir.dt.float32, tag="w1sb")
            nc.sync.dma_start(out=w1_sb[:, 0, :], in_=w1t[0:H, :])
            nc.sync.dma_start(out=w1_sb[:, 1, :], in_=w1t[H:2 * H, :])
            b0_sb = const.tile([1, H], mybir.dt.float32, tag="b0sb")
            nc.sync.dma_start(out=b0_sb[:], in_=b0r.ap())
            b1_sb = const.tile([1, H], mybir.dt.float32, tag="b1sb")
            nc.sync.dma_start(out=b1_sb[:], in_=b1r.ap())

            def layer_tail(acc_psum, dest_ap):
                hh = work.tile([P, H], mybir.dt.float32, tag="relu")
                nc.scalar.activation(out=hh[:], in_=acc_psum,
                                     func=mybir.ActivationFunctionType.Relu)
                sq = work.tile([P, H], mybir.dt.float32, tag="sq")
                ssq = work.tile([P, 1], mybir.dt.float32, tag="ssq")
                nc.scalar.activation(out=sq[:], in_=hh[:],
                                     func=mybir.ActivationFunctionType.Square,
                                     accum_out=ssq[:])
                nrm = work.tile([P, 1], mybir.dt.float32, tag="nrm")
                nc.scalar.activation(out=nrm[:], in_=ssq[:],
                                     func=mybir.ActivationFunctionType.Sqrt,
                                     bias=eps[:])
                rn = work.tile([P, 1], mybir.dt.float32, tag="rn")
                nc.vector.reciprocal(out=rn[:], in_=nrm[:])
                nc.vector.tensor_scalar_mul(dest_ap, hh[:], rn[:])

            def transpose_to_sb(src_ap, tag):
                pt = psum_t.tile([P, P], mybir.dt.float32, tag="tp")
                nc.tensor.transpose(out=pt[:], in_=src_ap, identity=ident[:])
                sb = work.tile([P, P], mybir.dt.float32, tag=tag)
                nc.vector.tensor_copy(out=sb[:], in_=pt[:])
                return sb

            def c_and_compute(t, ci):
                nonlocal qi
                fin = finp.tile([P, T * 26, F], mybir.dt.float32, tag="fin")
                for a in (0, 1):
                    h = 2 * t + a
                    stgv = stg[h][:]
                    for b in range(NCB):
                        nidx = 1024 if b < NCB - 1 else NPOS - 1024 * (NCB - 1)
                        nch = nidx // P
                        mstart = a * HCH + b * 8
                        nc.gpsimd.dma_gather(
                            out_ap=fin[:, mstart:mstart + nch, :], in_ap=stgv,
                            idxs_ap=ci[:, (a * NCB + b) * CW:
                                       (a * NCB + b) * CW + nidx // 16],
                            num_idxs=nidx, num_idxs_reg=nidx, elem_size=F,
                            queue_num=qi % 4, single_packet=True)
                        qi += 1

                h1slab = slab.tile([P, T, H], mybir.dt.float32, tag="h1")
                for s in range(T):
                    ns = work.tile([P, F], mybir.dt.float32, tag="ns")
                    nc.vector.tensor_reduce(
                        out=ns[:],
                        in_=fin[:, s * 26 + 1:(s + 1) * 26, :].rearrange("p j f -> p f j"),
                        axis=mybir.AxisListType.X, op=mybir.AluOpType.add,
                    )
                    xa = transpose_to_sb(fin[:, s * 26, :], "xa")
                    xb = transpose_to_sb(ns[:], "xb")
                    acc = psum_a.tile([P, H], mybir.dt.float32, tag="acc")
                    nc.tensor.matmul(acc[:], lhsT=xa[:], rhs=w0_sb[:, 0, :], start=True, stop=False)
                    nc.tensor.matmul(acc[:], lhsT=xb[:], rhs=w0_sb[:, 1, :], start=False, stop=False)
                    nc.tensor.matmul(acc[:], lhsT=ones[:1, :], rhs=b0_sb[:1, :], start=False, stop=True)
                    layer_tail(acc[:], h1slab[:, s, :])

                ns2 = work.tile([P, H], mybir.dt.float32, tag="ns2")
                nc.vector.tensor_reduce(
                    out=ns2[:], in_=h1slab[:, 1:, :].rearrange("p s f -> p f s"),
                    axis=mybir.AxisListType.X, op=mybir.AluOpType.add,
                )
                x2a = transpose_to_sb(h1slab[:, 0, :], "xa")
                x2b = transpose_to_sb(ns2[:], "xb")
                acc2 = psum_a.tile([P, H], mybir.dt.float32, tag="acc")
                nc.tensor.matmul(acc2[:], lhsT=x2a[:], rhs=w1_sb[:, 0, :], start=True, stop=False)
                nc.tensor.matmul(acc2[:], lhsT=x2b[:], rhs=w1_sb[:, 1, :], start=False, stop=False)
                nc.tensor.matmul(acc2[:], lhsT=ones[:1, :], rhs=b1_sb[:1, :], start=False, stop=True)
                zt = outp.tile([P, H], mybir.dt.float32, tag="z")
                layer_tail(acc2[:], zt[:])
                nc.sync.dma_start(out=zout[t * P:(t + 1) * P, :], in_=zt[:])

            prev = None
            for t in range(NTILES):
                ai = aip.tile([P, 2 * NW * AW], mybir.dt.int16, tag="ai")
                nc.sync.dma_start(out=ai[:], in_=aidx_d[:, 2 * t * NW * AW:
                                                        2 * (t + 1) * NW * AW])
                ci = cip.tile([P, 2 * NCB * CW], mybir.dt.int16, tag="ci")
                nc.sync.dma_start(out=ci[:], in_=cidx_d[:, 2 * t * NCB * CW:
                                                        2 * (t + 1) * NCB * CW])
                for a in (0, 1):
                    h = 2 * t + a
                    for wi in range(NW):
                        g = stgp.tile([P, W_BUD // P, F], mybir.dt.float32, tag="g")
                        nc.gpsimd.dma_gather(
                            out_ap=g[:], in_ap=feat[wi << 15:(wi + 1) << 15, :],
                            idxs_ap=ai[:, (a * NW + wi) * AW:(a * NW + wi + 1) * AW],
                            num_idxs=W_BUD, num_idxs_reg=W_BUD, elem_size=F,
                            queue_num=qi % 4, single_packet=True)
                        qi += 1
                        nc.sync.dma_start(
                            out=stg[h][wi * W_BUD:(wi + 1) * W_BUD, :], in_=g[:])
                if prev is not None:
                    c_and_compute(*prev)
                prev = (t, ci)
            c_and_compute(*prev)

    nc.compile()
    return nc


def _kernel_bass_v2(features, W0, b0, W1, b1, nodes, nbr1, nbr0, trace=False):
    from concourse.bass_utils import run_bass_kernel_spmd
    features, w0t, w1t, b0, b1, lvl1, nbr0f = _prep_host(
        features, W0, b0, W1, b1, nodes, nbr1, nbr0)
    featpad = np.zeros((NPADROWS, F), np.float32)
    featpad[:N] = features
    nbr03 = nbr0f.reshape(B, T, S0)
    in_maps = []
    for c in range(NC):
        sl = slice(c * BC, (c + 1) * BC)
        aidx, cidx = _plan_v2(lvl1[sl].astype(np.int64),
                              nbr03[sl].astype(np.int64))
        if aidx is None:
            raise RuntimeError("v2 plan overflow; use v1")
        in_maps.append(dict(
            features=featpad, w0t=w0t, w1t=w1t, b0r=b0, b1r=b1,
            aidx=aidx, cidx=cidx,
        ))
    if "nc2" not in _CACHE:
        _CACHE["nc2"] = _build_nc_v2()
    res = run_bass_kernel_spmd(_CACHE["nc2"], in_maps,
                               core_ids=list(range(NC)), trace=trace)
    out = np.concatenate([res.results[c]["zout"] for c in range(NC)], axis=0)
    if trace:
        return out, res
    return out


# ------------------------------------------------------------- jax fallback
def _kernel_jax(features, W0, b0, W1, b1, nodes, nbr1, nbr0):
    import jax
    import jax.numpy as jnp

    features, w0t, w1t, b0, b1, lvl1, nbr0f = _prep_host(
        features, W0, b0, W1, b1, nodes, nbr1, nbr0)
    w0s, w0n = w0t[:F], w0t[F:]
    w1s, w1n = w1t[:H], w1t[H:]

    def l2n(h):
        n = jnp.linalg.norm(h, axis=-1, keepdims=True)
        return h / jnp.maximum(n, 1e-12)

    def fwd(feat, w0s, w0n, b0, w1s, w1n, b1, lvl1c, nbr0c):
        h0s = feat[lvl1c]
        h0n = feat[nbr0c].reshape(BC, T, S0, F).sum(2)
        h1 = l2n(jax.nn.relu(h0s @ w0s + h0n @ w0n + b0))
        return l2n(jax.nn.relu(h1[:, 0, :] @ w1s + h1[:, 1:, :].sum(1) @ w1n + b1))

    if "jax_fns" not in _CACHE:
        _CACHE["jax_fns"] = {}
    devs = jax.devices()[:NC]
    outs = []
    for c, d in enumerate(devs):
        sl = slice(c * BC, (c + 1) * BC)
        args = (jax.device_put(features, d),
                jax.device_put(w0s, d), jax.device_put(w0n, d), jax.device_put(b0[0], d),
                jax.device_put(w1s, d), jax.device_put(w1n, d), jax.device_put(b1[0], d),
                jax.device_put(np.ascontiguousarray(lvl1[sl]), d),
                jax.device_put(np.ascontiguousarray(nbr0f[sl]), d))
        if d not in _CACHE["jax_fns"]:
            _CACHE["jax_fns"][d] = jax.jit(fwd)
        outs.append(_CACHE["jax_fns"][d](*args))
    return np.concatenate([np.asarray(jax.block_until_ready(o)) for o in outs], axis=0)


def kernel(features, W0, b0, W1, b1, nodes, nbr1, nbr0):
    try:
        return _kernel_bass(features, W0, b0, W1, b1, nodes, nbr1, nbr0)
    except Exception:
        import traceback
        traceback.print_exc()
        return _kernel_jax(features, W0, b0, W1, b1, nodes, nbr1, nbr0)



# revision 2
# speedup vs baseline: 1.7561x; 1.7561x over previous
"""GraphSAGE-style 2-layer minibatch forward (gnn_message_passing) on 8 trn2
NeuronCores — self-contained bass/Tile kernel.

Problem (hardcoded shapes):
    features [1_000_000, 128] f32, W0/W1 [128, 256] f32, b0/b1 [128] f32,
    nodes [4096] int, nbr1 [4096, 10] int, nbr0 [4096, 11, 25] int
    out z [4096, 128] f32

Strategy: data-parallel over the 4096 target nodes (512 per core, 4 tiles of
128); the feature table and the small weights are replicated per core, so no
cross-core communication is needed.  Per (tile, slot) the kernel issues 26
indirect-DMA gathers (one table row per SBUF partition, 128 rows / 64 KB per
instruction — the only vector-indirect form the SWDGE ucode supports),
round-robined over 4 SWDGE queues.  The 25 neighbor rows are summed on DVE
(strided tensor_reduce), self/neighbor-sum tiles transposed on PE, and both
dense layers run as K-chunked PE matmuls with the bias added via a rank-1
matmul.  The mean-over-neighbors is folded into pre-scaled weight halves:
    relu(W0 @ [x_self ; mean_j x_j]) == relu(x_self @ W0s.T + (sum_j x_j) @ (W0n/25).T)
The l2-normalize uses 1/sqrt(ssq + 1e-24), which matches the reference's
h / max(||h||, 1e-12) for every reachable input.
"""
import numpy as np

N = 1_000_000
F = 128
H = 128
B = 4096
S1 = 10
S0 = 25
NC = 8
BC = B // NC          # 512 targets per core
T = 1 + S1            # 11 level-1 slots
P = 128
NTILES = BC // P      # 4

_QUEUES = ["qPoolDynamic", "qPoolDynamic1", "qPoolDynamic2", "qPoolDynamic3"]
import os as _os
_SINGLE_PACKET = _os.environ.get("K_SINGLE_PACKET", "1") == "1"
_NQUEUES = int(_os.environ.get("K_NQUEUES", "4"))
_NBUFS = int(_os.environ.get("K_NBUFS", "5"))

_CACHE = {}


# ----------------------------------------------------------------- bass path
def _build_nc():
    import concourse.bass as bass
    import concourse.tile as tile
    from contextlib import ExitStack
    from concourse import bacc, mybir
    from concourse.masks import make_identity

    nc = bacc.Bacc("TRN2", target_bir_lowering=False, debug=False,
                   num_swdge_queues=_NQUEUES)
    feat = nc.dram_tensor("features", [N, F], mybir.dt.float32, kind="ExternalInput")
    w0t = nc.dram_tensor("w0t", [2 * F, H], mybir.dt.float32, kind="ExternalInput")
    w1t = nc.dram_tensor("w1t", [2 * H, H], mybir.dt.float32, kind="ExternalInput")
    b0r = nc.dram_tensor("b0r", [1, H], mybir.dt.float32, kind="ExternalInput")
    b1r = nc.dram_tensor("b1r", [1, H], mybir.dt.float32, kind="ExternalInput")
    idx_self = nc.dram_tensor("idx_self", [BC, T], mybir.dt.int32, kind="ExternalInput")
    idx_nbr = nc.dram_tensor("idx_nbr", [BC, T * S0], mybir.dt.int32, kind="ExternalInput")
    zout = nc.dram_tensor("zout", [BC, H], mybir.dt.float32, kind="ExternalOutput")

    qi = 0

    def gather(out_ap, off_ap):
        nonlocal qi
        ins_obj = nc.gpsimd.indirect_dma_start(
            out=out_ap, out_offset=None, in_=feat[:],
            in_offset=bass.IndirectOffsetOnAxis(ap=off_ap, axis=0),
        )
        ins_obj.ins.queue = _QUEUES[qi % _NQUEUES]
        ins_obj.ins.single_packet = _SINGLE_PACKET
        qi += 1
        return ins_obj

    with tile.TileContext(nc) as tc:
        with ExitStack() as ctx:
            const = ctx.enter_context(tc.tile_pool(name="const", bufs=1))
            idxp = ctx.enter_context(tc.tile_pool(name="idxp", bufs=2))
            nbrp = ctx.enter_context(tc.tile_pool(name="nbrp", bufs=_NBUFS))
            selfp = ctx.enter_context(tc.tile_pool(name="selfp", bufs=_NBUFS))
            work = ctx.enter_context(tc.tile_pool(name="work", bufs=6))
            slab = ctx.enter_context(tc.tile_pool(name="slab", bufs=2))
            outp = ctx.enter_context(tc.tile_pool(name="outp", bufs=2))
            psum_t = ctx.enter_context(tc.tile_pool(name="psum_t", bufs=4, space="PSUM"))
            psum_a = ctx.enter_context(tc.tile_pool(name="psum_a", bufs=4, space="PSUM"))

            ident = const.tile([P, P], mybir.dt.float32)
            make_identity(nc, ident[:])
            ones = const.tile([1, P], mybir.dt.float32)
            nc.vector.memset(ones[:], 1.0)
            eps = const.tile([P, 1], mybir.dt.float32, tag="eps")
            nc.vector.memset(eps[:], 1e-24)
            w0_sb = const.tile([P, 2, H], mybir.dt.float32, tag="w0sb")
            nc.sync.dma_start(out=w0_sb[:, 0, :], in_=w0t[0:F, :])
            nc.sync.dma_start(out=w0_sb[:, 1, :], in_=w0t[F:2 * F, :])
            w1_sb = const.tile([P, 2, H], mybir.dt.float32, tag="w1sb")
            nc.sync.dma_start(out=w1_sb[:, 0, :], in_=w1t[0:H, :])
            nc.sync.dma_start(out=w1_sb[:, 1, :], in_=w1t[H:2 * H, :])
            b0_sb = const.tile([1, H], mybir.dt.float32, tag="b0sb")
            nc.sync.dma_start(out=b0_sb[:], in_=b0r.ap())
            b1_sb = const.tile([1, H], mybir.dt.float32, tag="b1sb")
            nc.sync.dma_start(out=b1_sb[:], in_=b1r.ap())

            def layer_tail(acc_psum, dest_ap):
                h = work.tile([P, H], mybir.dt.float32, tag="relu")
                nc.scalar.activation(out=h[:], in_=acc_psum,
                                     func=mybir.ActivationFunctionType.Relu)
                sq = work.tile([P, H], mybir.dt.float32, tag="sq")
                ssq = work.tile([P, 1], mybir.dt.float32, tag="ssq")
                nc.scalar.activation(out=sq[:], in_=h[:],
                                     func=mybir.ActivationFunctionType.Square,
                                     accum_out=ssq[:])
                nrm = work.tile([P, 1], mybir.dt.float32, tag="nrm")
                nc.scalar.activation(out=nrm[:], in_=ssq[:],
                                     func=mybir.ActivationFunctionType.Sqrt,
                                     bias=eps[:])
                rn = work.tile([P, 1], mybir.dt.float32, tag="rn")
                nc.vector.reciprocal(out=rn[:], in_=nrm[:])
                nc.vector.tensor_scalar_mul(dest_ap, h[:], rn[:])

            def transpose_to_sb(src_ap, tag):
                pt = psum_t.tile([P, P], mybir.dt.float32, tag="tp")
                nc.tensor.transpose(out=pt[:], in_=src_ap, identity=ident[:])
                sb = work.tile([P, P], mybir.dt.float32, tag=tag)
                nc.vector.tensor_copy(out=sb[:], in_=pt[:])
                return sb

            import concourse.mybir as mybir_  # alias for closures above
            for t in range(NTILES):
                rows = slice(t * P, (t + 1) * P)
                idxs_t = idxp.tile([P, T], mybir.dt.int32, tag="idxs")
                nc.sync.dma_start(out=idxs_t[:], in_=idx_self[rows, :])
                idxn_t = idxp.tile([P, T * S0], mybir.dt.int32, tag="idxn")
                nc.sync.dma_start(out=idxn_t[:], in_=idx_nbr[rows, :])

                h1slab = slab.tile([P, T, H], mybir.dt.float32, tag="h1")

                for s in range(T):
                    sf = selfp.tile([P, F], mybir.dt.float32, tag="sf")
                    gather(sf[:], idxs_t[:, s:s + 1])
                    g = nbrp.tile([P, S0, F], mybir.dt.float32, tag="g")
                    for j in range(S0):
                        gather(g[:, j, :], idxn_t[:, s * S0 + j:s * S0 + j + 1])

                    ns = work.tile([P, F], mybir.dt.float32, tag="ns")
                    nc.vector.tensor_reduce(
                        out=ns[:], in_=g[:].rearrange("p j f -> p f j"),
                        axis=mybir.AxisListType.X, op=mybir.AluOpType.add,
                    )
                    xa = transpose_to_sb(sf[:], "xa")
                    xb = transpose_to_sb(ns[:], "xb")
                    acc = psum_a.tile([P, H], mybir.dt.float32, tag="acc")
                    nc.tensor.matmul(acc[:], lhsT=xa[:], rhs=w0_sb[:, 0, :], start=True, stop=False)
                    nc.tensor.matmul(acc[:], lhsT=xb[:], rhs=w0_sb[:, 1, :], start=False, stop=False)
                    nc.tensor.matmul(acc[:], lhsT=ones[:1, :], rhs=b0_sb[:1, :], start=False, stop=True)
                    layer_tail(acc[:], h1slab[:, s, :])

                ns2 = work.tile([P, H], mybir.dt.float32, tag="ns2")
                nc.vector.tensor_reduce(
                    out=ns2[:], in_=h1slab[:, 1:, :].rearrange("p s f -> p f s"),
                    axis=mybir.AxisListType.X, op=mybir.AluOpType.add,
                )
                x2a = transpose_to_sb(h1slab[:, 0, :], "xa")
                x2b = transpose_to_sb(ns2[:], "xb")
                acc2 = psum_a.tile([P, H], mybir.dt.float32, tag="acc")
                nc.tensor.matmul(acc2[:], lhsT=x2a[:], rhs=w1_sb[:, 0, :], start=True, stop=False)
                nc.tensor.matmul(acc2[:], lhsT=x2b[:], rhs=w1_sb[:, 1, :], start=False, stop=False)
                nc.tensor.matmul(acc2[:], lhsT=ones[:1, :], rhs=b1_sb[:1, :], start=False, stop=True)
                zt = outp.tile([P, H], mybir.dt.float32, tag="z")
                layer_tail(acc2[:], zt[:])
                nc.sync.dma_start(out=zout[rows, :], in_=zt[:])

    nc.compile()
    return nc


def _prep_host(features, W0, b0, W1, b1, nodes, nbr1, nbr0):
    features = np.ascontiguousarray(np.asarray(features, dtype=np.float32))
    W0 = np.asarray(W0, dtype=np.float32)
    W1 = np.asarray(W1, dtype=np.float32)
    b0 = np.asarray(b0, dtype=np.float32).reshape(1, H)
    b1 = np.asarray(b1, dtype=np.float32).reshape(1, H)
    lvl1 = np.concatenate([np.asarray(nodes).reshape(B, 1),
                           np.asarray(nbr1).reshape(B, S1)], axis=1).astype(np.int32)
    nbr0f = np.asarray(nbr0).reshape(B, T * S0).astype(np.int32)
    w0t = np.ascontiguousarray(np.concatenate([W0[:, :F].T, W0[:, F:].T / S0], axis=0))
    w1t = np.ascontiguousarray(np.concatenate([W1[:, :H].T, W1[:, H:].T / S1], axis=0))
    return features, w0t, w1t, b0, b1, lvl1, nbr0f


def _kernel_bass(features, W0, b0, W1, b1, nodes, nbr1, nbr0, trace=False):
    from concourse.bass_utils import run_bass_kernel_spmd
    if "nc" not in _CACHE:
        _CACHE["nc"] = _build_nc()
    nc = _CACHE["nc"]
    features, w0t, w1t, b0, b1, lvl1, nbr0f = _prep_host(
        features, W0, b0, W1, b1, nodes, nbr1, nbr0)
    in_maps = []
    for c in range(NC):
        sl = slice(c * BC, (c + 1) * BC)
        in_maps.append(dict(
            features=features, w0t=w0t, w1t=w1t, b0r=b0, b1r=b1,
            idx_self=np.ascontiguousarray(lvl1[sl]),
            idx_nbr=np.ascontiguousarray(nbr0f[sl]),
        ))
    res = run_bass_kernel_spmd(nc, in_maps, core_ids=list(range(NC)), trace=trace)
    out = np.concatenate([res.results[c]["zout"] for c in range(NC)], axis=0)
    if trace:
        return out, res
    return out


# ----------------------------------------------------- v2: two-phase bounce
# Phase A: per (tile-half, 32K-row window) dma_gather of the half's
# row-sorted refs (padded to W_BUD) into SBUF, streamed to a compact HBM
# staging table. Phase C: dma_gather from the <=32K-row staging straight into
# the final [128, 286, F] compute layout (idx = host-computed staged
# positions). Compute identical to v1. ~390 Ant-gather instructions replace
# 1144 indirect ones on the Pool engine.
W_BUD = 768                # staged rows per (half, window) = 6 chunks
NW = 31                    # 31 windows cover rows [0, 1_015_808) >= N
HCH = T * 26 // 2          # 143 chunks per half-tile
NPOS = HCH * P             # 18304 final positions per half
STG_H = NW * W_BUD         # 23808 staged rows per half (int16-addressable)
NPADROWS = 1 << 20
AW = W_BUD // 16           # 48 idx words per window slot
CW = 64                    # idx words per C-block slot (1024/16)
NCB = 18                   # C blocks per half (17x1024 + 896)


def _wrap16(vals, width):
    a = np.zeros((16, width), np.int16)
    n = len(vals)
    a[np.arange(n) % 16, np.arange(n) // 16] = vals.astype(np.int16)
    return np.tile(a, (8, 1))


def _plan_v2(lvl1c, nbr0c3):
    """Per-core plan: (aidx [128, 8*NW*AW], cidx [128, 8*NCB*CW]) or None."""
    aidx = np.zeros((128, 8 * NW * AW), np.int16)
    cidx = np.zeros((128, 8 * NCB * CW), np.int16)
    for t in range(NTILES):
        trows = t * P + np.arange(P)
        for a in (0, 1):
            h = 2 * t + a
            ms = np.arange(a * HCH, (a + 1) * HCH)
            s_of, j_of = ms // 26, ms % 26
            r_self = lvl1c[trows[:, None], s_of[None, :]]
            r_nbr = nbr0c3[trows[:, None], s_of[None, :],
                           np.maximum(j_of - 1, 0)[None, :]]
            refs = np.where((j_of == 0)[None, :], r_self, r_nbr)   # [p, m]
            flat = refs.T.reshape(-1)                              # k = m*128+p
            order = np.argsort(flat, kind="stable")
            sref = flat[order]
            w = (sref >> 15).astype(np.int64)
            cnt = np.bincount(w, minlength=NW)
            if cnt.max() > W_BUD:
                return None, None
            starts = np.concatenate([[0], np.cumsum(cnt)])
            rank = np.arange(NPOS) - starts[w]
            # staged row within the window block follows the plain [p, c, f]
            # write order of the gather tile: token r -> row (r%128)*6 + r//128
            spos = w * W_BUD + (rank % P) * (W_BUD // P) + rank // P
            cpos = np.empty(NPOS, np.int64)
            cpos[order] = spos
            for wi in range(NW):
                loc = sref[starts[wi]:starts[wi + 1]] - (wi << 15)
                aidx[:, (h * NW + wi) * AW:(h * NW + wi + 1) * AW] = \
                    _wrap16(np.concatenate([loc, np.zeros(W_BUD - len(loc),
                                                          np.int64)]), AW)
            for b in range(NCB):
                nidx = 1024 if b < NCB - 1 else NPOS - 1024 * (NCB - 1)
                vals = cpos[b * 1024: b * 1024 + nidx]
                cidx[:, (h * NCB + b) * CW:(h * NCB + b) * CW + CW] = \
                    _wrap16(vals, CW)
    return aidx, cidx


def _build_nc_v2():
    import concourse.bass as bass
    import concourse.tile as tile
    from contextlib import ExitStack
    from concourse import bacc, mybir
    from concourse.masks import make_identity

    nc = bacc.Bacc("TRN2", target_bir_lowering=False, debug=False,
                   num_swdge_queues=4)
    feat = nc.dram_tensor("features", [NPADROWS, F], mybir.dt.float32, kind="ExternalInput")
    w0t = nc.dram_tensor("w0t", [2 * F, H], mybir.dt.float32, kind="ExternalInput")
    w1t = nc.dram_tensor("w1t", [2 * H, H], mybir.dt.float32, kind="ExternalInput")
    b0r = nc.dram_tensor("b0r", [1, H], mybir.dt.float32, kind="ExternalInput")
    b1r = nc.dram_tensor("b1r", [1, H], mybir.dt.float32, kind="ExternalInput")
    aidx_d = nc.dram_tensor("aidx", [128, 8 * NW * AW], mybir.dt.int16, kind="ExternalInput")
    cidx_d = nc.dram_tensor("cidx", [128, 8 * NCB * CW], mybir.dt.int16, kind="ExternalInput")
    stg = [nc.dram_tensor(f"stg{h}", [STG_H, F], mybir.dt.float32,
                          kind="Internal") for h in range(8)]
    zout = nc.dram_tensor("zout", [BC, H], mybir.dt.float32, kind="ExternalOutput")

    qi = 0

    with tile.TileContext(nc) as tc:
        with ExitStack() as ctx:
            const = ctx.enter_context(tc.tile_pool(name="const", bufs=1))
            aip = ctx.enter_context(tc.tile_pool(name="aip", bufs=1))
            cip = ctx.enter_context(tc.tile_pool(name="cip", bufs=2))
            stgp = ctx.enter_context(tc.tile_pool(name="stgp", bufs=4))
            finp = ctx.enter_context(tc.tile_pool(name="finp", bufs=1))
            work = ctx.enter_context(tc.tile_pool(name="work", bufs=6))
            slab = ctx.enter_context(tc.tile_pool(name="slab", bufs=2))
            outp = ctx.enter_context(tc.tile_pool(name="outp", bufs=2))
            psum_t = ctx.enter_context(tc.tile_pool(name="psum_t", bufs=4, space="PSUM"))
            psum_a = ctx.enter_context(tc.tile_pool(name="psum_a", bufs=4, space="PSUM"))

            ident = const.tile([P, P], mybir.dt.float32)
            make_identity(nc, ident[:])
            ones = const.tile([1, P], mybir.dt.float32)
            nc.vector.memset(ones[:], 1.0)
            eps = const.tile([P, 1], mybir.dt.float32, tag="eps")
            nc.vector.memset(eps[:], 1e-24)
            w0_sb = const.tile([P, 2, H], mybir.dt.float32, tag="w0sb")
            nc.sync.dma_start(out=w0_sb[:, 0, :], in_=w0t[0:F, :])
            nc.sync.dma_start(out=w0_sb[:, 1, :], in_=w0t[F:2 * F, :])
            w1_sb = const.tile([P, 2, H], mybir.dt.float32, tag="w1sb")
            nc.sync.dma_start(out=w1_sb[:, 0, :], in_=w1t[0:H, :])
            nc.sync.dma_start(out=w1_sb[:, 1, :], in_=w1t[H:2 * H, :])
            b0_sb = const.tile([1, H], mybir.dt.float32, tag="b0sb")
            nc.sync.dma_start(out=b0_sb[:], in_=b0r.ap())
            b1_sb = const.tile([1, H], mybir.dt.float32, tag="b1sb")
            nc.sync.dma_start(out=b1_sb[:], in_=b1r.ap())

            def layer_tail(acc_psum, dest_ap):
                hh = work.tile([P, H], mybir.dt.float32, tag="relu")
                nc.scalar.activation(out=hh[:], in_=acc_psum,
                                     func=mybir.ActivationFunctionType.Relu)
                sq = work.tile([P, H], mybir.dt.float32, tag="sq")
                ssq = work.tile([P, 1], mybir.dt.float32, tag="ssq")
                nc.scalar.activation(out=sq[:], in_=hh[:],
                                     func=mybir.ActivationFunctionType.Square,
                                     accum_out=ssq[:])
                nrm = work.tile([P, 1], mybir.dt.float32, tag="nrm")
                nc.scalar.activation(out=nrm[:], in_=ssq[:],
                                     func=mybir.ActivationFunctionType.Sqrt,
                                     bias=eps[:])
                rn = work.tile([P, 1], mybir.dt.float32, tag="rn")
                nc.vector.reciprocal(out=rn[:], in_=nrm[:])
                nc.vector.tensor_scalar_mul(dest_ap, hh[:], rn[:])

            def transpose_to_sb(src_ap, tag):
                pt = psum_t.tile([P, P], mybir.dt.float32, tag="tp")
                nc.tensor.transpose(out=pt[:], in_=src_ap, identity=ident[:])
                sb = work.tile([P, P], mybir.dt.float32, tag=tag)
                nc.vector.tensor_copy(out=sb[:], in_=pt[:])
                return sb

            def c_and_compute(t, ci):
                nonlocal qi
                fin = finp.tile([P, T * 26, F], mybir.dt.float32, tag="fin")
                for a in (0, 1):
                    h = 2 * t + a
                    stgv = stg[h][:]
                    for b in range(NCB):
                        nidx = 1024 if b < NCB - 1 else NPOS - 1024 * (NCB - 1)
                        nch = nidx // P
                        mstart = a * HCH + b * 8
                        nc.gpsimd.dma_gather(
                            out_ap=fin[:, mstart:mstart + nch, :], in_ap=stgv,
                            idxs_ap=ci[:, (a * NCB + b) * CW:
                                       (a * NCB + b) * CW + nidx // 16],
                            num_idxs=nidx, num_idxs_reg=nidx, elem_size=F,
                            queue_num=qi % 4, single_packet=True)
                        qi += 1

                h1slab = slab.tile([P, T, H], mybir.dt.float32, tag="h1")
                for s in range(T):
                    ns = work.tile([P, F], mybir.dt.float32, tag="ns")
                    nc.vector.tensor_reduce(
                        out=ns[:],
                        in_=fin[:, s * 26 + 1:(s + 1) * 26, :].rearrange("p j f -> p f j"),
                        axis=mybir.AxisListType.X, op=mybir.AluOpType.add,
                    )
                    xa = transpose_to_sb(fin[:, s * 26, :], "xa")
                    xb = transpose_to_sb(ns[:], "xb")
                    acc = psum_a.tile([P, H], mybir.dt.float32, tag="acc")
                    nc.tensor.matmul(acc[:], lhsT=xa[:], rhs=w0_sb[:, 0, :], start=True, stop=False)
                    nc.tensor.matmul(acc[:], lhsT=xb[:], rhs=w0_sb[:, 1, :], start=False, stop=False)
                    nc.tensor.matmul(acc[:], lhsT=ones[:1, :], rhs=b0_sb[:1, :], start=False, stop=True)
                    layer_tail(acc[:], h1slab[:, s, :])

                ns2 = work.tile([P, H], mybir.dt.float32, tag="ns2")
                nc.vector.tensor_reduce(
                    out=ns2[:], in_=h1slab[:, 1:, :].rearrange("p s f -> p f s"),
                    axis=mybir.AxisListType.X, op=mybir.AluOpType.add,
                )
                x2a = transpose_to_sb(h1slab[:, 0, :], "xa")
                x2b = transpose_to_sb(ns2[:], "xb")
                acc2 = psum_a.tile([P, H], mybir.dt.float32, tag="acc")
                nc.tensor.matmul(acc2[:], lhsT=x2a[:], rhs=w1_sb[:, 0, :], start=True, stop=False)
                nc.tensor.matmul(acc2[:], lhsT=x2b[:], rhs=w1_sb[:, 1, :], start=False, stop=False)
                nc.tensor.matmul(acc2[:], lhsT=ones[:1, :], rhs=b1_sb[:1, :], start=False, stop=True)
                zt = outp.tile([P, H], mybir.dt.float32, tag="z")
                layer_tail(acc2[:], zt[:])
                nc.sync.dma_start(out=zout[t * P:(t + 1) * P, :], in_=zt[:])

            prev = None
            for t in range(NTILES):
                ai = aip.tile([P, 2 * NW * AW], mybir.dt.int16, tag="ai")
                nc.sync.dma_start(out=ai[:], in_=aidx_d[:, 2 * t * NW * AW:
                                                        2 * (t + 1) * NW * AW])
                ci = cip.tile([P, 2 * NCB * CW], mybir.dt.int16, tag="ci")
                nc.sync.dma_start(out=ci[:], in_=cidx_d[:, 2 * t * NCB * CW:
                                                        2 * (t + 1) * NCB * CW])
                for a in (0, 1):
                    h = 2 * t + a
                    for wi in range(NW):
                        g = stgp.tile([P, W_BUD // P, F], mybir.dt.float32, tag="g")
                        nc.gpsimd.dma_gather(
                            out_ap=g[:], in_ap=feat[wi << 15:(wi + 1) << 15, :],
                            idxs_ap=ai[:, (a * NW + wi) * AW:(a * NW + wi + 1) * AW],
                            num_idxs=W_BUD, num_idxs_reg=W_BUD, elem_size=F,
                            queue_num=qi % 4, single_packet=True)
                        qi += 1
                        nc.sync.dma_start(
                            out=stg[h][wi * W_BUD:(wi + 1) * W_BUD, :], in_=g[:])
                if prev is not None:
                    c_and_compute(*prev)
                prev = (t, ci)
            c_and_compute(*prev)

    nc.compile()
    return nc


def _kernel_bass_v2(features, W0, b0, W1, b1, nodes, nbr1, nbr0, trace=False):
    from concourse.bass_utils import run_bass_kernel_spmd
    features, w0t, w1t, b0, b1, lvl1, nbr0f = _prep_host(
        features, W0, b0, W1, b1, nodes, nbr1, nbr0)
    featpad = np.zeros((NPADROWS, F), np.float32)
    featpad[:N] = features
    nbr03 = nbr0f.reshape(B, T, S0)
    in_maps = []
    for c in range(NC):
        sl = slice(c * BC, (c + 1) * BC)
        aidx, cidx = _plan_v2(lvl1[sl].astype(np.int64),
                              nbr03[sl].astype(np.int64))
        if aidx is None:
            raise RuntimeError("v2 plan overflow; use v1")
        in_maps.append(dict(
            features=featpad, w0t=w0t, w1t=w1t, b0r=b0, b1r=b1,
            aidx=aidx, cidx=cidx,
        ))
    if "nc2" not in _CACHE:
        _CACHE["nc2"] = _build_nc_v2()
    res = run_bass_kernel_spmd(_CACHE["nc2"], in_maps,
                               core_ids=list(range(NC)), trace=trace)
    out = np.concatenate([res.results[c]["zout"] for c in range(NC)], axis=0)
    if trace:
        return out, res
    return out




# ----------------------------------------------------------- v4: v1 in bf16
# Same single-phase architecture and Pool instruction stream as v1 (the
# 26x128-row indirect gathers are the proven Pool floor), but the table,
# weights and all PE/DVE traffic are bf16: half the HBM bytes, 2x DVE
# reduce throughput, bf16 PE matmuls.  Norms stay f32 (rel err ~4e-3).
import ml_dtypes as _mld

_NBUFS4 = int(_os.environ.get("K_NBUFS4", "8"))


def _build_nc_v4():
    import concourse.bass as bass
    import concourse.tile as tile
    from contextlib import ExitStack
    from concourse import bacc, mybir
    from concourse.masks import make_identity

    nc = bacc.Bacc("TRN2", target_bir_lowering=False, debug=False,
                   num_swdge_queues=_NQUEUES)
    feat = nc.dram_tensor("features", [N, F], mybir.dt.bfloat16, kind="ExternalInput")
    w0t = nc.dram_tensor("w0t", [2 * F, H], mybir.dt.bfloat16, kind="ExternalInput")
    w1t = nc.dram_tensor("w1t", [2 * H, H], mybir.dt.bfloat16, kind="ExternalInput")
    b0r = nc.dram_tensor("b0r", [1, H], mybir.dt.bfloat16, kind="ExternalInput")
    b1r = nc.dram_tensor("b1r", [1, H], mybir.dt.bfloat16, kind="ExternalInput")
    idx_self = nc.dram_tensor("idx_self", [BC, T], mybir.dt.int32, kind="ExternalInput")
    idx_nbr = nc.dram_tensor("idx_nbr", [BC, T * S0], mybir.dt.int32, kind="ExternalInput")
    zout = nc.dram_tensor("zout", [BC, H], mybir.dt.float32, kind="ExternalOutput")

    qi = 0

    def gather(out_ap, off_ap):
        nonlocal qi
        ins_obj = nc.gpsimd.indirect_dma_start(
            out=out_ap, out_offset=None, in_=feat[:],
            in_offset=bass.IndirectOffsetOnAxis(ap=off_ap, axis=0),
        )
        ins_obj.ins.queue = _QUEUES[qi % _NQUEUES]
        ins_obj.ins.single_packet = _SINGLE_PACKET
        qi += 1
        return ins_obj

    with tile.TileContext(nc) as tc:
        from contextlib import ExitStack as _ES
        ctx0 = _ES()
        ctx0.enter_context(nc.allow_low_precision("bf16 pipeline; 2e-2 gate"))
        with ctx0, _ES() as ctx:
            const = ctx.enter_context(tc.tile_pool(name="const", bufs=1))
            idxp = ctx.enter_context(tc.tile_pool(name="idxp", bufs=2))
            nbrp = ctx.enter_context(tc.tile_pool(name="nbrp", bufs=_NBUFS4))
            selfp = ctx.enter_context(tc.tile_pool(name="selfp", bufs=_NBUFS4))
            work = ctx.enter_context(tc.tile_pool(name="work", bufs=4))
            slab = ctx.enter_context(tc.tile_pool(name="slab", bufs=2))
            outp = ctx.enter_context(tc.tile_pool(name="outp", bufs=2))
            psum_t = ctx.enter_context(tc.tile_pool(name="psum_t", bufs=4, space="PSUM"))
            psum_a = ctx.enter_context(tc.tile_pool(name="psum_a", bufs=4, space="PSUM"))

            ident = const.tile([P, P], mybir.dt.bfloat16)
            make_identity(nc, ident[:])
            ones = const.tile([1, P], mybir.dt.bfloat16)
            nc.vector.memset(ones[:], 1.0)
            eps = const.tile([P, 1], mybir.dt.float32, tag="eps")
            nc.vector.memset(eps[:], 1e-24)
            w0_sb = const.tile([P, 2, H], mybir.dt.bfloat16, tag="w0sb")
            nc.sync.dma_start(out=w0_sb[:, 0, :], in_=w0t[0:F, :])
            nc.sync.dma_start(out=w0_sb[:, 1, :], in_=w0t[F:2 * F, :])
            w1_sb = const.tile([P, 2, H], mybir.dt.bfloat16, tag="w1sb")
            nc.sync.dma_start(out=w1_sb[:, 0, :], in_=w1t[0:H, :])
            nc.sync.dma_start(out=w1_sb[:, 1, :], in_=w1t[H:2 * H, :])
            b0_sb = const.tile([1, H], mybir.dt.bfloat16, tag="b0sb")
            nc.sync.dma_start(out=b0_sb[:], in_=b0r.ap())
            b1_sb = const.tile([1, H], mybir.dt.bfloat16, tag="b1sb")
            nc.sync.dma_start(out=b1_sb[:], in_=b1r.ap())

            def layer_tail(acc_psum, dest_ap):
                h = work.tile([P, H], mybir.dt.float32, tag="relu")
                nc.scalar.activation(out=h[:], in_=acc_psum,
                                     func=mybir.ActivationFunctionType.Relu)
                sq = work.tile([P, H], mybir.dt.float32, tag="sq")
                ssq = work.tile([P, 1], mybir.dt.float32, tag="ssq")
                nc.scalar.activation(out=sq[:], in_=h[:],
                                     func=mybir.ActivationFunctionType.Square,
                                     accum_out=ssq[:])
                nrm = work.tile([P, 1], mybir.dt.float32, tag="nrm")
                nc.scalar.activation(out=nrm[:], in_=ssq[:],
                                     func=mybir.ActivationFunctionType.Sqrt,
                                     bias=eps[:])
                rn = work.tile([P, 1], mybir.dt.float32, tag="rn")
                nc.vector.reciprocal(out=rn[:], in_=nrm[:])
                nc.vector.tensor_scalar_mul(dest_ap, h[:], rn[:])

            def transpose_to_sb(src_ap, tag):
                pt = psum_t.tile([P, P], mybir.dt.bfloat16, tag="tp")
                nc.tensor.transpose(out=pt[:], in_=src_ap, identity=ident[:])
                sb = work.tile([P, P], mybir.dt.bfloat16, tag=tag)
                nc.vector.tensor_copy(out=sb[:], in_=pt[:])
                return sb

            for t in range(NTILES):
                rows = slice(t * P, (t + 1) * P)
                idxs_t = idxp.tile([P, T], mybir.dt.int32, tag="idxs")
                nc.sync.dma_start(out=idxs_t[:], in_=idx_self[rows, :])
                idxn_t = idxp.tile([P, T * S0], mybir.dt.int32, tag="idxn")
                nc.sync.dma_start(out=idxn_t[:], in_=idx_nbr[rows, :])

                h1slab = slab.tile([P, T, H], mybir.dt.bfloat16, tag="h1")

                for s in range(T):
                    sf = selfp.tile([P, F], mybir.dt.bfloat16, tag="sf")
                    gather(sf[:], idxs_t[:, s:s + 1])
                    g = nbrp.tile([P, S0, F], mybir.dt.bfloat16, tag="g")
                    for j in range(S0):
                        gather(g[:, j, :], idxn_t[:, s * S0 + j:s * S0 + j + 1])

                    ns = work.tile([P, F], mybir.dt.bfloat16, tag="ns")
                    nc.vector.tensor_reduce(
                        out=ns[:], in_=g[:].rearrange("p j f -> p f j"),
                        axis=mybir.AxisListType.X, op=mybir.AluOpType.add,
                    )
                    xa = transpose_to_sb(sf[:], "xa")
                    xb = transpose_to_sb(ns[:], "xb")
                    acc = psum_a.tile([P, H], mybir.dt.float32, tag="acc")
                    nc.tensor.matmul(acc[:], lhsT=xa[:], rhs=w0_sb[:, 0, :], start=True, stop=False)
                    nc.tensor.matmul(acc[:], lhsT=xb[:], rhs=w0_sb[:, 1, :], start=False, stop=False)
                    nc.tensor.matmul(acc[:], lhsT=ones[:1, :], rhs=b0_sb[:1, :], start=False, stop=True)
                    layer_tail(acc[:], h1slab[:, s, :])

                ns2 = work.tile([P, H], mybir.dt.bfloat16, tag="ns2")
                nc.vector.tensor_reduce(
                    out=ns2[:], in_=h1slab[:, 1:, :].rearrange("p s f -> p f s"),
                    axis=mybir.AxisListType.X, op=mybir.AluOpType.add,
                )
                x2a = transpose_to_sb(h1slab[:, 0, :], "xa")
                x2b = transpose_to_sb(ns2[:], "xb")
                acc2 = psum_a.tile([P, H], mybir.dt.float32, tag="acc")
                nc.tensor.matmul(acc2[:], lhsT=x2a[:], rhs=w1_sb[:, 0, :], start=True, stop=False)
                nc.tensor.matmul(acc2[:], lhsT=x2b[:], rhs=w1_sb[:, 1, :], start=False, stop=False)
                nc.tensor.matmul(acc2[:], lhsT=ones[:1, :], rhs=b1_sb[:1, :], start=False, stop=True)
                zt = outp.tile([P, H], mybir.dt.float32, tag="z")
                layer_tail(acc2[:], zt[:])
                nc.sync.dma_start(out=zout[rows, :], in_=zt[:])

    nc.compile()
    return nc


def _prep_host_v4(features, W0, b0, W1, b1, nodes, nbr1, nbr0):
    featb = np.asarray(features, dtype=np.float32).astype(_mld.bfloat16)
    W0 = np.asarray(W0, dtype=np.float32)
    W1 = np.asarray(W1, dtype=np.float32)
    w0t = np.ascontiguousarray(
        np.concatenate([W0[:, :F].T, W0[:, F:].T / S0], axis=0)).astype(_mld.bfloat16)
    w1t = np.ascontiguousarray(
        np.concatenate([W1[:, :H].T, W1[:, H:].T / S1], axis=0)).astype(_mld.bfloat16)
    b0r = np.asarray(b0, dtype=np.float32).reshape(1, H).astype(_mld.bfloat16)
    b1r = np.asarray(b1, dtype=np.float32).reshape(1, H).astype(_mld.bfloat16)
    lvl1 = np.concatenate([np.asarray(nodes).reshape(B, 1),
                           np.asarray(nbr1).reshape(B, S1)], axis=1).astype(np.int32)
    nbr0f = np.asarray(nbr0).reshape(B, T * S0).astype(np.int32)
    return featb, w0t, w1t, b0r, b1r, lvl1, nbr0f


def _kernel_bass_v4(features, W0, b0, W1, b1, nodes, nbr1, nbr0, trace=False):
    from concourse.bass_utils import run_bass_kernel_spmd
    if "nc4" not in _CACHE:
        _CACHE["nc4"] = _build_nc_v4()
    nc = _CACHE["nc4"]
    featb, w0t, w1t, b0r, b1r, lvl1, nbr0f = _prep_host_v4(
        features, W0, b0, W1, b1, nodes, nbr1, nbr0)
    in_maps = []
    for c in range(NC):
        sl = slice(c * BC, (c + 1) * BC)
        in_maps.append(dict(
            features=featb, w0t=w0t, w1t=w1t, b0r=b0r, b1r=b1r,
            idx_self=np.ascontiguousarray(lvl1[sl]),
            idx_nbr=np.ascontiguousarray(nbr0f[sl]),
        ))
    res = run_bass_kernel_spmd(nc, in_maps, core_ids=list(range(NC)), trace=trace)
    out = np.concatenate([res.results[c]["zout"] for c in range(NC)], axis=0)
    if trace:
        return out, res
    return out


# ------------------------------------------------------------- jax fallback
def _kernel_jax(features, W0, b0, W1, b1, nodes, nbr1, nbr0):
    import jax
    import jax.numpy as jnp

    features, w0t, w1t, b0, b1, lvl1, nbr0f = _prep_host(
        features, W0, b0, W1, b1, nodes, nbr1, nbr0)
    w0s, w0n = w0t[:F], w0t[F:]
    w1s, w1n = w1t[:H], w1t[H:]

    def l2n(h):
        n = jnp.linalg.norm(h, axis=-1, keepdims=True)
        return h / jnp.maximum(n, 1e-12)

    def fwd(feat, w0s, w0n, b0, w1s, w1n, b1, lvl1c, nbr0c):
        h0s = feat[lvl1c]
        h0n = feat[nbr0c].reshape(BC, T, S0, F).sum(2)
        h1 = l2n(jax.nn.relu(h0s @ w0s + h0n @ w0n + b0))
        return l2n(jax.nn.relu(h1[:, 0, :] @ w1s + h1[:, 1:, :].sum(1) @ w1n + b1))

    if "jax_fns" not in _CACHE:
        _CACHE["jax_fns"] = {}
    devs = jax.devices()[:NC]
    outs = []
    for c, d in enumerate(devs):
        sl = slice(c * BC, (c + 1) * BC)
        args = (jax.device_put(features, d),
                jax.device_put(w0s, d), jax.device_put(w0n, d), jax.device_put(b0[0], d),
                jax.device_put(w1s, d), jax.device_put(w1n, d), jax.device_put(b1[0], d),
                jax.device_put(np.ascontiguousarray(lvl1[sl]), d),
                jax.device_put(np.ascontiguousarray(nbr0f[sl]), d))
        if d not in _CACHE["jax_fns"]:
            _CACHE["jax_fns"][d] = jax.jit(fwd)
        outs.append(_CACHE["jax_fns"][d](*args))
    return np.concatenate([np.asarray(jax.block_until_ready(o)) for o in outs], axis=0)


def kernel(features, W0, b0, W1, b1, nodes, nbr1, nbr0):
    try:
        return _kernel_bass(features, W0, b0, W1, b1, nodes, nbr1, nbr0)
    except Exception:
        import traceback
        traceback.print_exc()
        return _kernel_jax(features, W0, b0, W1, b1, nodes, nbr1, nbr0)

